# revision 4
# baseline (speedup 1.0000x reference)
"""LoRA-MHSA Trainium2 kernel.

Data-parallel over batch B=8 (one sample per NeuronCore). The per-sample LoRA
adapters are folded into the base weights on the host (exact algebra:
x@(W + (a/r)B@A).T == x@W.T + (a/r)(x@A.T)@B.T), so the device kernel is a
plain MHSA with per-core weights:
  qkv = x @ Wqkv_eff.T + b_qkv ; 16-head SDPA over T=1024, dh=64 ;
  out = y @ Wp_eff.T + b_p

All matmul operands are bf16 (full PE rate, halves DMA + SBUF); PSUM
accumulation stays fp32. Biases are added during the PSUM->SBUF drain copies
(host-replicated bias tiles), costing no extra engine time.

Layout: activations channel-major ([C, T]) so q/k head slabs feed the scores
matmul directly and PV feeds the projection without transposes. v is natural
[T, C] with a per-head 65th ones-column so the PV matmul emits the softmax
denominator in PSUM row 64. Softmax needs no max-subtraction: scores are O(1)
by construction.

Schedule: x streams in 8 chunks on the SP DMA queue while all weights stream
on the Pool queue (v-phase starts ~1us in); the q/k build of head-pair hp is
interleaved step-by-step with the attention of pair hp-1 (with a small build
head-start) so Exp (ACT) hides under build matmuls; scores for both heads of
a pair land in one 2-bank PSUM group and are exponentiated by a single fused
ACT instruction; ys drains to SBUF immediately to recycle PSUM banks; the
projection weights prefetch during attention so the final GEMM starts
stall-free.
"""

import sys
from itertools import zip_longest

sys.path.insert(0, "/opt/trn_rl_repo")

import numpy as np
import ml_dtypes

import concourse.bass as bass
import concourse.tile as tile
from concourse import bacc, mybir
from concourse.bass_utils import run_bass_kernel_spmd

T = 1024
C = 1024
C3 = 3072
H = 16
DH = 64
RANK = 8
ALPHA_OVER_RANK = 1.0 / 8.0
SM_SCALE = 0.125  # 1/sqrt(dh)
NCORES = 8

F32 = mybir.dt.float32
BF16 = mybir.dt.bfloat16
EXP = mybir.ActivationFunctionType.Exp
NPBF16 = ml_dtypes.bfloat16

ts = bass.ts

TT = T // 128     # 8 t tiles
TCH = T // 512    # 2 t chunks (psum free dim)
CINT = C // 128   # 8 contraction tiles
NPAIR = H // 2    # 8 head pairs


def _build():
    nc = bacc.Bacc("TRN2", target_bir_lowering=False, debug=False)

    xT_d = nc.dram_tensor("xT", [C, T], BF16, kind="ExternalInput")
    wqkvT_d = nc.dram_tensor("wqkvT", [C, C3], BF16, kind="ExternalInput")
    wpT_d = nc.dram_tensor("wpT", [C, C], BF16, kind="ExternalInput")
    bqk_d = nc.dram_tensor("bqk", [128, H], F32, kind="ExternalInput")
    bv_d = nc.dram_tensor("bv", [128, C], F32, kind="ExternalInput")
    bo_d = nc.dram_tensor("bo", [128, C], F32, kind="ExternalInput")
    out_d = nc.dram_tensor("out", [T, C], F32, kind="ExternalOutput")

    with tile.TileContext(nc) as tc:
      with tc.tile_pool(name="res", bufs=1) as res:
        xT = res.tile([128, CINT, T], BF16, tag="xT")
        vv = res.tile([128, TT, H, DH + 1], BF16, tag="vv")
        yt = res.tile([128, CINT, T], BF16, tag="yt")
        wpa = res.tile([128, CINT, C], BF16, tag="wpa")
        # q/k weight slabs: [g] covers 4 consecutive 128-col parts.
        # g0: q cols 0-511 (pairs 0-3), g1: q 512-1023, g2: k 1024-1535,
        # g3: k 1536-2047.
        wqk = [
            res.tile([128, CINT, 512], BF16, tag=f"wqk{g}", name=f"wqk{g}")
            for g in range(4)
        ]
        bqk = res.tile([128, H], F32, tag="bqk")
        bv = res.tile([128, H, DH], F32, tag="bv")
        bo = res.tile([128, C], F32, tag="bo")

        # ---- DMA streams: x chunks on SP, all weights on Pool ----
        for ci in range(CINT):
            nc.sync.dma_start(out=xT[:, ci, :], in_=xT_d[ts(ci, 128), :])
        nc.sync.dma_start(out=bo[:], in_=bo_d[:])
        nc.sync.dma_start(out=bqk[:], in_=bqk_d[:])

        with tc.tile_pool(name="wvp", bufs=2) as wvp:
            wvv = []
            for cch in range(2):
                w = wvp.tile([128, CINT, 512], BF16, tag="wv", name=f"wvv{cch}")
                for ci in range(CINT):
                    nc.gpsimd.dma_start(
                        out=w[:, ci, :],
                        in_=wqkvT_d[
                            ts(ci, 128), 2048 + cch * 512 : 2560 + cch * 512
                        ],
                    )
                wvv.append(w)
            nc.gpsimd.dma_start(
                out=bv[:], in_=bv_d.rearrange("p (h d) -> p h d", d=DH)
            )
            for g in range(4):
                nc.gpsimd.dma_start(
                    out=wqk[g][:],
                    in_=wqkvT_d[:, ts(g, 512)].rearrange("(n p) c -> p n c", p=128),
                )
            nc.gpsimd.dma_start(
                out=wpa[:], in_=wpT_d.rearrange("(n p) c -> p n c", p=128)
            )

            nc.vector.memset(vv[:, :, :, DH : DH + 1], 1.0)

            # ---- phase 1: v = x @ W_v.T + b_v -> vv [T(keys), h, 65] ----
            with tc.tile_pool(name="vps", bufs=8, space="PSUM") as vps:
                for cch in range(2):
                    for ttg in range(2):
                        pvq = [vps.tile([128, 512], F32, tag="pv", name="pv")
                               for _ in range(4)]
                        for ci in range(CINT):
                            for j in range(4):
                                tt = ttg * 4 + j
                                nc.tensor.matmul(
                                    pvq[j][:], xT[:, ci, ts(tt, 128)],
                                    wvv[cch][:, ci, :],
                                    start=(ci == 0), stop=(ci == CINT - 1),
                                )
                        for j in range(4):
                            tt = ttg * 4 + j
                            nc.vector.tensor_add(
                                vv[:, tt, cch * 8 : cch * 8 + 8, 0:DH],
                                pvq[j][:].rearrange("p (h d) -> p h d", d=DH),
                                bv[:, cch * 8 : cch * 8 + 8, :],
                            )

            # ---- interleaved: qk build for pair hp + attention pair hp-1 ----
            with tc.tile_pool(name="qkpool", bufs=3) as qkpool, \
                 tc.tile_pool(name="qkps", bufs=2, space="PSUM") as qkps, \
                 tc.tile_pool(name="att", bufs=3) as att, \
                 tc.tile_pool(name="sps", bufs=2, space="PSUM") as spsp, \
                 tc.tile_pool(name="yps", bufs=2, space="PSUM") as ypsp:

                qktiles = {}

                def build_steps(hp_i):
                    qkt = qkpool.tile([128, 2, T], BF16, tag="qkt", name="qkt")
                    qktiles[hp_i] = qkt
                    for part in range(2):          # 0: q, 1: k
                        g = 2 * part + hp_i // 4
                        col = (hp_i % 4) * 128
                        bcol = hp_i + 8 * part
                        pqs = [
                            qkps.tile([128, 512], F32, tag="pq", name="pq")
                            for _ in range(TCH)
                        ]
                        for ci in range(CINT):
                            def step(ci=ci, pqs=pqs, g=g, col=col):
                                for tch in range(TCH):
                                    nc.tensor.matmul(
                                        pqs[tch][:],
                                        wqk[g][:, ci, col : col + 128],
                                        xT[:, ci, ts(tch, 512)],
                                        start=(ci == 0), stop=(ci == CINT - 1),
                                    )
                            yield step
                        def fin(part=part, pqs=pqs, qkt=qkt, bcol=bcol):
                            for tch in range(TCH):
                                nc.vector.tensor_scalar_add(
                                    qkt[:, part, ts(tch, 512)], pqs[tch][:],
                                    bqk[:, bcol : bcol + 1],
                                )
                        yield fin

                def att_steps(hp_i):
                    qkt = qktiles[hp_i]
                    for tqc in range(TCH):
                        ys = [
                            ypsp.tile([DH + 1, 512], F32, tag="yp", name="yp")
                            for _ in range(2)
                        ]
                        pend = {}

                        def scores_exp(tkt, tqc=tqc, qkt=qkt, pend=pend):
                            sp = spsp.tile([128, 2, 512], F32, tag="sp", name="sp")
                            for sub in range(2):
                                po = sub * DH
                                nc.tensor.matmul(
                                    sp[:, sub, :],
                                    qkt[po : po + DH, 1, ts(tkt, 128)],
                                    qkt[po : po + DH, 0, ts(tqc, 512)],
                                    start=True, stop=True,
                                )
                            e = att.tile([128, 2, 512], BF16, tag="e", name="e")
                            nc.scalar.activation(e[:], sp[:], EXP, scale=SM_SCALE)
                            pend[tkt] = e

                        def pv(tkt, ys=ys, hp_i=hp_i, pend=pend):
                            e = pend.pop(tkt)
                            for sub in range(2):
                                h = 2 * hp_i + sub
                                nc.tensor.matmul(
                                    ys[sub][:], vv[:, tkt, h, :], e[:, sub, :],
                                    start=(tkt == 0), stop=(tkt == TT - 1),
                                )

                        # one-step software pipeline: PV trails scores/exp so
                        # the in-order PE never waits on a same-step Exp
                        for tkt in range(TT):
                            def step(tkt=tkt):
                                scores_exp(tkt)
                                if tkt > 0:
                                    pv(tkt - 1)
                            yield step
                        def flush(ys=ys, tqc=tqc, hp_i=hp_i):
                            pv(TT - 1)
                        yield flush
                        def norm(tqc=tqc, ys=ys, hp_i=hp_i):
                            for sub in range(2):
                                po = sub * DH
                                # drain ys to SBUF promptly to recycle the
                                # PSUM bank; normalize from the SBUF copy
                                ysc = att.tile([DH + 1, 512], BF16, tag="ysc",
                                               name="ysc", bufs=4)
                                nc.vector.tensor_copy(ysc[:], ys[sub][:])
                                r = att.tile([1, 512], BF16, tag="r", name="r",
                                             bufs=2)
                                with nc.allow_low_precision(reason="softmax recip"):
                                    nc.vector.reciprocal(r[:], ysc[DH : DH + 1, :])
                                rb = att.tile([DH, 512], BF16, tag="rb",
                                              name="rb", bufs=2)
                                nc.gpsimd.partition_broadcast(rb[:], r[:])
                                nc.vector.tensor_mul(
                                    yt[po : po + DH, hp_i, ts(tqc, 512)],
                                    ysc[0:DH, :], rb[:],
                                )
                        yield norm

                for hp_i in range(NPAIR):
                    bgen = build_steps(hp_i)
                    if hp_i > 0:
                        # head start: finish the build a little before the
                        # previous pair's attention ends so the qkt drain
                        # copies are done when the next attention starts
                        for _ in range(3):
                            s = next(bgen, None)
                            if s is not None:
                                s()
                        agen = att_steps(hp_i - 1)
                    else:
                        agen = iter(())
                    for bs, as_ in zip_longest(bgen, agen):
                        if bs is not None:
                            bs()
                        if as_ is not None:
                            as_()
                for as_ in att_steps(NPAIR - 1):
                    as_()

            # ---- phase 3: out = y @ W_p.T + b_p (natural [T, C]) ----
            with tc.tile_pool(name="ops", bufs=3, space="PSUM") as ops, \
                 tc.tile_pool(name="ot", bufs=3) as otp:
                for tt in range(TT):
                    pos = ops.tile([128, 2, 512], F32, tag="po", name="po")
                    for ci in range(CINT):
                        for cch in range(2):
                            nc.tensor.matmul(
                                pos[:, cch, :], yt[:, ci, ts(tt, 128)],
                                wpa[:, ci, ts(cch, 512)],
                                start=(ci == 0), stop=(ci == CINT - 1),
                            )
                    ot = otp.tile([128, C], F32, tag="ot", name="ot")
                    nc.vector.tensor_add(
                        ot[:].rearrange("p (a c) -> p a c", a=2), pos[:],
                        bo[:].rearrange("p (a c) -> p a c", a=2),
                    )
                    nc.sync.dma_start(out=out_d[ts(tt, 128), :], in_=ot[:])

    nc.compile()
    return nc


_NC_CACHE = {}


def _in_maps(inputs):
    x = np.asarray(inputs["x"], dtype=np.float32)
    sid = np.asarray(inputs["subject_id"]).astype(np.int64)
    W_qkv = np.asarray(inputs["W_qkv"], dtype=np.float32)
    b_qkv = np.asarray(inputs["b_qkv"], dtype=np.float32)
    A1 = np.asarray(inputs["A1"], dtype=np.float32)
    B1 = np.asarray(inputs["B1"], dtype=np.float32)
    W_p = np.asarray(inputs["W_p"], dtype=np.float32)
    b_p = np.asarray(inputs["b_p"], dtype=np.float32)
    A2 = np.asarray(inputs["A2"], dtype=np.float32)
    B2 = np.asarray(inputs["B2"], dtype=np.float32)

    bqk = np.ascontiguousarray(b_qkv[:2048].reshape(H, 128).T)
    bv = np.ascontiguousarray(
        np.broadcast_to(b_qkv[2048:3072], (128, C)).astype(np.float32)
    )
    bo = np.ascontiguousarray(np.broadcast_to(b_p, (128, C)).astype(np.float32))

    in_maps = []
    for b in range(NCORES):
        s = int(sid[b])
        W1 = W_qkv + ALPHA_OVER_RANK * (B1[s] @ A1[s])
        Wp = W_p + ALPHA_OVER_RANK * (B2[s] @ A2[s])
        in_maps.append(
            {
                "xT": np.ascontiguousarray(x[b].T).astype(NPBF16),
                "wqkvT": np.ascontiguousarray(W1.T).astype(NPBF16),
                "wpT": np.ascontiguousarray(Wp.T).astype(NPBF16),
                "bqk": bqk,
                "bv": bv,
                "bo": bo,
            }
        )
    return in_maps


def kernel(**inputs):
    if "nc" not in _NC_CACHE:
        _NC_CACHE["nc"] = _build()
    nc = _NC_CACHE["nc"]

    res = run_bass_kernel_spmd(nc, _in_maps(inputs), core_ids=list(range(NCORES)))
    out = np.stack([r["out"] for r in res.results], axis=0)
    return out.astype(np.float32)


# revision 18
# speedup vs baseline: 1.0172x; 1.0172x over previous
"""LoRA-MHSA Trainium2 kernel.

Data-parallel over batch B=8 (one sample per NeuronCore). The per-sample LoRA
adapters are folded into the base weights on the host (exact algebra:
x@(W + (a/r)B@A).T == x@W.T + (a/r)(x@A.T)@B.T), so the device kernel is a
plain MHSA with per-core weights:
  qkv = x @ Wqkv_eff.T + b_qkv ; 16-head SDPA over T=1024, dh=64 ;
  out = y @ Wp_eff.T + b_p

All matmul operands are bf16 (full PE rate, halves DMA + SBUF); PSUM
accumulation stays fp32. Biases are added during the PSUM->SBUF drain copies
(host-replicated bias tiles), costing no extra engine time.

Layout: activations channel-major ([C, T]) so q/k head slabs feed the scores
matmul directly and PV feeds the projection without transposes. v is natural
[T, C] with a per-head 65th ones-column so the PV matmul emits the softmax
denominator in PSUM row 64. Softmax needs no max-subtraction: scores are O(1)
by construction.

Schedule: x streams in 8 chunks on the SP DMA queue while all weights stream
on the Pool queue (v-phase starts ~1us in); the q/k build of head-pair hp is
interleaved step-by-step with the attention of pair hp-1 (with a small build
head-start) so Exp (ACT) hides under build matmuls; scores for both heads of
a pair land in one 2-bank PSUM group and are exponentiated by a single fused
ACT instruction; ys drains to SBUF immediately to recycle PSUM banks; the
projection weights prefetch during attention so the final GEMM starts
stall-free.
"""

import sys
from itertools import zip_longest

sys.path.insert(0, "/opt/trn_rl_repo")

import numpy as np
import ml_dtypes

import concourse.bass as bass
import concourse.tile as tile
from concourse import bacc, mybir
from concourse.bass_utils import run_bass_kernel_spmd

T = 1024
C = 1024
C3 = 3072
H = 16
DH = 64
RANK = 8
ALPHA_OVER_RANK = 1.0 / 8.0
SM_SCALE = 0.125  # 1/sqrt(dh)
NCORES = 8

F32 = mybir.dt.float32
BF16 = mybir.dt.bfloat16
EXP = mybir.ActivationFunctionType.Exp
NPBF16 = ml_dtypes.bfloat16

ts = bass.ts

TT = T // 128     # 8 t tiles
TCH = T // 512    # 2 t chunks (psum free dim)
CINT = C // 128   # 8 contraction tiles
NPAIR = H // 2    # 8 head pairs


def _build():
    nc = bacc.Bacc("TRN2", target_bir_lowering=False, debug=False)

    xT_d = nc.dram_tensor("xT", [C, T], BF16, kind="ExternalInput")
    wqkvT_d = nc.dram_tensor("wqkvT", [C, C3], BF16, kind="ExternalInput")
    wpT_d = nc.dram_tensor("wpT", [C, C], BF16, kind="ExternalInput")
    bqk_d = nc.dram_tensor("bqk", [128, H], F32, kind="ExternalInput")
    bv_d = nc.dram_tensor("bv", [128, C], F32, kind="ExternalInput")
    bo_d = nc.dram_tensor("bo", [128, C], F32, kind="ExternalInput")
    out_d = nc.dram_tensor("out", [T, C], F32, kind="ExternalOutput")

    with tile.TileContext(nc) as tc:
      with tc.tile_pool(name="res", bufs=1) as res:
        xT = res.tile([128, CINT, T], BF16, tag="xT")
        vv = res.tile([128, TT, H, DH + 1], BF16, tag="vv")
        yt = res.tile([128, CINT, T], BF16, tag="yt")
        wpa = res.tile([128, CINT, C], BF16, tag="wpa")
        # q/k weight slabs: [g] covers 4 consecutive 128-col parts.
        # g0: q cols 0-511 (pairs 0-3), g1: q 512-1023, g2: k 1024-1535,
        # g3: k 1536-2047.
        wqk = [
            res.tile([128, CINT, 512], BF16, tag=f"wqk{g}", name=f"wqk{g}")
            for g in range(4)
        ]
        bqk = res.tile([128, H], F32, tag="bqk")
        bv = res.tile([128, H, DH], F32, tag="bv")
        bo = res.tile([128, C], F32, tag="bo")

        # ---- DMA streams: x chunks on SP, all weights on Pool ----
        # first chunk arrives in 256-col slivers so the very first stationary
        # load (xT[:, 0, 0:128]) lands ~0.6us earlier
        for s in range(4):
            nc.sync.dma_start(
                out=xT[:, 0, ts(s, 256)], in_=xT_d[0:128, ts(s, 256)]
            )
        for ci in range(1, CINT):
            nc.sync.dma_start(out=xT[:, ci, :], in_=xT_d[ts(ci, 128), :])
        nc.sync.dma_start(out=bo[:], in_=bo_d[:])
        nc.sync.dma_start(out=bqk[:], in_=bqk_d[:])

        with tc.tile_pool(name="wvp", bufs=2) as wvp:
            wvv = []
            for cch in range(2):
                w = wvp.tile([128, CINT, 512], BF16, tag="wv", name=f"wvv{cch}")
                for ci in range(CINT):
                    nc.gpsimd.dma_start(
                        out=w[:, ci, :],
                        in_=wqkvT_d[
                            ts(ci, 128), 2048 + cch * 512 : 2560 + cch * 512
                        ],
                    )
                wvv.append(w)
            nc.gpsimd.dma_start(
                out=bv[:], in_=bv_d.rearrange("p (h d) -> p h d", d=DH)
            )
            for g in range(4):
                nc.gpsimd.dma_start(
                    out=wqk[g][:],
                    in_=wqkvT_d[:, ts(g, 512)].rearrange("(n p) c -> p n c", p=128),
                )
            nc.gpsimd.dma_start(
                out=wpa[:], in_=wpT_d.rearrange("(n p) c -> p n c", p=128)
            )

            nc.vector.memset(vv[:, :, :, DH : DH + 1], 1.0)

            # ---- phase 1: v = x @ W_v.T + b_v -> vv [T(keys), h, 65] ----
            with tc.tile_pool(name="vps", bufs=8, space="PSUM") as vps:
                for cch in range(2):
                    for ttg in range(2):
                        last_q = cch == 1 and ttg == 1
                        pvq = [vps.tile([128, 512], F32, tag="pv", name="pv")
                               for _ in range(4)]
                        if last_q:
                            # pre-write the bias into PSUM so the drains are
                            # plain copies that split across ACT+DVE: the
                            # v->attention pool-switch barrier then waits
                            # ~1.3us instead of 2.6us of serial DVE drains
                            for j in range(4):
                                nc.vector.tensor_copy(
                                    pvq[j][:].rearrange("p (h d) -> p h d", d=DH),
                                    bv[:, cch * 8 : cch * 8 + 8, :],
                                )
                        for ci in range(CINT):
                            for j in range(4):
                                tt = ttg * 4 + j
                                nc.tensor.matmul(
                                    pvq[j][:], xT[:, ci, ts(tt, 128)],
                                    wvv[cch][:, ci, :],
                                    start=(ci == 0 and not last_q),
                                    stop=(ci == CINT - 1),
                                )
                        for j in range(4):
                            tt = ttg * 4 + j
                            dst = vv[:, tt, cch * 8 : cch * 8 + 8, 0:DH]
                            src = pvq[j][:].rearrange("p (h d) -> p h d", d=DH)
                            if last_q:
                                if j < 2:
                                    nc.scalar.copy(dst, src)
                                else:
                                    nc.vector.tensor_copy(dst, src)
                            else:
                                nc.vector.tensor_add(
                                    dst, src, bv[:, cch * 8 : cch * 8 + 8, :]
                                )

            # ---- interleaved: qk build for pair hp + attention pair hp-1 ----
            with tc.tile_pool(name="qkpool", bufs=3) as qkpool, \
                 tc.tile_pool(name="qkps", bufs=2, space="PSUM") as qkps, \
                 tc.tile_pool(name="att", bufs=3) as att, \
                 tc.tile_pool(name="sps", bufs=2, space="PSUM") as spsp, \
                 tc.tile_pool(name="yps", bufs=2, space="PSUM") as ypsp:

                qktiles = {}

                def build_steps(hp_i):
                    qkt = qkpool.tile([128, 2, T], BF16, tag="qkt", name="qkt")
                    qktiles[hp_i] = qkt
                    for part in range(2):          # 0: q, 1: k
                        g = 2 * part + hp_i // 4
                        col = (hp_i % 4) * 128
                        bcol = hp_i + 8 * part
                        pqs = [
                            qkps.tile([128, 512], F32, tag="pq", name="pq")
                            for _ in range(TCH)
                        ]
                        for ci in range(CINT):
                            def step(ci=ci, pqs=pqs, g=g, col=col):
                                for tch in range(TCH):
                                    nc.tensor.matmul(
                                        pqs[tch][:],
                                        wqk[g][:, ci, col : col + 128],
                                        xT[:, ci, ts(tch, 512)],
                                        start=(ci == 0), stop=(ci == CINT - 1),
                                    )
                            yield step
                        def fin(part=part, pqs=pqs, qkt=qkt, bcol=bcol):
                            # split the two drains across ACT and DVE so they
                            # run in parallel: the next part's PSUM-bank reuse
                            # (and the next pair's scores) wait ~0.7us less
                            nc.scalar.activation(
                                qkt[:, part, 0:512], pqs[0][:],
                                mybir.ActivationFunctionType.Identity,
                                bias=bqk[:, bcol : bcol + 1],
                            )
                            nc.vector.tensor_scalar_add(
                                qkt[:, part, 512:1024], pqs[1][:],
                                bqk[:, bcol : bcol + 1],
                            )
                        yield fin

                def att_steps(hp_i):
                    qkt = qktiles[hp_i]
                    for tqc in range(TCH):
                        ys = [
                            ypsp.tile([DH + 1, 512], F32, tag="yp", name="yp")
                            for _ in range(2)
                        ]
                        pend = {}

                        def scores_exp(tkt, tqc=tqc, qkt=qkt, pend=pend):
                            sp = spsp.tile([128, 2, 512], F32, tag="sp", name="sp")
                            for sub in range(2):
                                po = sub * DH
                                nc.tensor.matmul(
                                    sp[:, sub, :],
                                    qkt[po : po + DH, 1, ts(tkt, 128)],
                                    qkt[po : po + DH, 0, ts(tqc, 512)],
                                    start=True, stop=True,
                                )
                            e = att.tile([128, 2, 512], BF16, tag="e", name="e")
                            nc.scalar.activation(e[:], sp[:], EXP, scale=SM_SCALE)
                            pend[tkt] = e

                        def pv(tkt, ys=ys, hp_i=hp_i, pend=pend):
                            e = pend.pop(tkt)
                            for sub in range(2):
                                h = 2 * hp_i + sub
                                nc.tensor.matmul(
                                    ys[sub][:], vv[:, tkt, h, :], e[:, sub, :],
                                    start=(tkt == 0), stop=(tkt == TT - 1),
                                )

                        # one-step software pipeline: PV trails scores/exp so
                        # the in-order PE never waits on a same-step Exp
                        for tkt in range(TT):
                            def step(tkt=tkt):
                                scores_exp(tkt)
                                if tkt > 0:
                                    pv(tkt - 1)
                            yield step
                        def flush(ys=ys, tqc=tqc, hp_i=hp_i):
                            pv(TT - 1)
                        yield flush
                        def norm(tqc=tqc, ys=ys, hp_i=hp_i):
                            for sub in range(2):
                                po = sub * DH
                                # drain ys to SBUF promptly to recycle the
                                # PSUM bank; normalize from the SBUF copy
                                ysc = att.tile([DH + 1, 512], BF16, tag="ysc",
                                               name="ysc", bufs=4)
                                nc.vector.tensor_copy(ysc[:], ys[sub][:])
                                r = att.tile([1, 512], BF16, tag="r", name="r",
                                             bufs=2)
                                with nc.allow_low_precision(reason="softmax recip"):
                                    nc.vector.reciprocal(r[:], ysc[DH : DH + 1, :])
                                rb = att.tile([DH, 512], BF16, tag="rb",
                                              name="rb", bufs=2)
                                nc.gpsimd.partition_broadcast(rb[:], r[:])
                                nc.vector.tensor_mul(
                                    yt[po : po + DH, hp_i, ts(tqc, 512)],
                                    ysc[0:DH, :], rb[:],
                                )
                        yield norm

                def proj_tile(tt, pos_ap, otp, ci_lo=0, cch_major=False,
                              pos_sl=None):
                    # pos_ap: callable cch -> [128, 512] PSUM AP
                    # pos_sl: callable (cch, lo, hi) -> [128, hi-lo] PSUM AP
                    ot = otp.tile([128, C], F32, tag="ot", name="ot")

                    def drain(cch):
                        nc.vector.tensor_add(
                            ot[:, ts(cch, 512)], pos_ap(cch),
                            bo[:, ts(cch, 512)],
                        )
                        nc.gpsimd.dma_start(
                            out=out_d[ts(tt, 128), ts(cch, 512)],
                            in_=ot[:, ts(cch, 512)],
                        )

                    if cch_major:
                        # close + drain the first half while the second
                        # half's matmuls still run: shortens the final tail
                        for cch in range(2):
                            for ci in range(ci_lo, CINT):
                                nc.tensor.matmul(
                                    pos_ap(cch), yt[:, ci, ts(tt, 128)],
                                    wpa[:, ci, ts(cch, 512)],
                                    start=(ci == 0), stop=(ci == CINT - 1),
                                )
                            if cch == 0:
                                drain(0)
                        # final drain in quarters, stores split across the
                        # Pool and SP DMA queues so they overlap
                        for q in range(2):
                            sl = slice(512 + q * 256, 768 + q * 256)
                            nc.vector.tensor_add(
                                ot[:, sl], pos_sl(1, q * 256, q * 256 + 256),
                                bo[:, sl],
                            )
                            eng = nc.gpsimd if q == 0 else nc.sync
                            eng.dma_start(
                                out=out_d[ts(tt, 128), sl], in_=ot[:, sl]
                            )
                    else:
                        for ci in range(ci_lo, CINT):
                            for cch in range(2):
                                nc.tensor.matmul(
                                    pos_ap(cch), yt[:, ci, ts(tt, 128)],
                                    wpa[:, ci, ts(cch, 512)],
                                    start=(ci == 0), stop=(ci == CINT - 1),
                                )
                        for cch in range(2):
                            drain(cch)

                for hp_i in range(NPAIR):
                    bgen = build_steps(hp_i)
                    if hp_i > 0:
                        # head start: finish the build a little before the
                        # previous pair's attention ends so the qkt drain
                        # copies are done when the next attention starts
                        for _ in range(3):
                            s = next(bgen, None)
                            if s is not None:
                                s()
                        agen = att_steps(hp_i - 1)
                    else:
                        agen = iter(())
                    for bs, as_ in zip_longest(bgen, agen):
                        if bs is not None:
                            bs()
                        if as_ is not None:
                            as_()

                # final pair's attention has no build to hide Exp under —
                # interleave the ci 0..6 accumulation of the first output
                # tile into it, reusing the (idle) build PSUM banks
                with tc.tile_pool(name="ot0", bufs=1) as otp0:
                    pe0 = [qkps.tile([128, 512], F32, tag="pq", name="pq")
                           for _ in range(2)]

                    def proj_early_steps():
                        for ci in range(CINT - 1):
                            def step(ci=ci):
                                for cch in range(2):
                                    nc.tensor.matmul(
                                        pe0[cch][:], yt[:, ci, 0:128],
                                        wpa[:, ci, ts(cch, 512)],
                                        start=(ci == 0), stop=False,
                                    )
                            yield step

                    for as_, ps in zip_longest(att_steps(NPAIR - 1),
                                               proj_early_steps()):
                        if as_ is not None:
                            as_()
                        if ps is not None:
                            ps()
                    proj_tile(0, lambda cch: pe0[cch][:], otp0,
                              ci_lo=CINT - 1)

            # ---- phase 3: out = y @ W_p.T + b_p (natural [T, C]) ----
            with tc.tile_pool(name="ops", bufs=3, space="PSUM") as ops, \
                 tc.tile_pool(name="ot", bufs=3) as otp:
                for tt in range(1, TT - 1):
                    pos = ops.tile([128, 2, 512], F32, tag="po", name="po")
                    proj_tile(tt, lambda cch, pos=pos: pos[:, cch, :], otp)
                # last tile: two independent single-bank tiles so the second
                # half's accumulation doesn't serialize behind the first
                # half's drain (tile-level WAR)
                pl = [ops.tile([128, 512], F32, tag=f"pl{i}", name=f"pl{i}",
                               bufs=1)
                      for i in range(2)]
                proj_tile(TT - 1, lambda cch: pl[cch][:], otp, cch_major=True,
                          pos_sl=lambda cch, lo, hi: pl[cch][:, lo:hi])

    nc.compile()
    return nc


_NC_CACHE = {}


def _in_maps(inputs):
    x = np.asarray(inputs["x"], dtype=np.float32)
    sid = np.asarray(inputs["subject_id"]).astype(np.int64)
    W_qkv = np.asarray(inputs["W_qkv"], dtype=np.float32)
    b_qkv = np.asarray(inputs["b_qkv"], dtype=np.float32)
    A1 = np.asarray(inputs["A1"], dtype=np.float32)
    B1 = np.asarray(inputs["B1"], dtype=np.float32)
    W_p = np.asarray(inputs["W_p"], dtype=np.float32)
    b_p = np.asarray(inputs["b_p"], dtype=np.float32)
    A2 = np.asarray(inputs["A2"], dtype=np.float32)
    B2 = np.asarray(inputs["B2"], dtype=np.float32)

    bqk = np.ascontiguousarray(b_qkv[:2048].reshape(H, 128).T)
    bv = np.ascontiguousarray(
        np.broadcast_to(b_qkv[2048:3072], (128, C)).astype(np.float32)
    )
    bo = np.ascontiguousarray(np.broadcast_to(b_p, (128, C)).astype(np.float32))

    in_maps = []
    for b in range(NCORES):
        s = int(sid[b])
        W1 = W_qkv + ALPHA_OVER_RANK * (B1[s] @ A1[s])
        Wp = W_p + ALPHA_OVER_RANK * (B2[s] @ A2[s])
        in_maps.append(
            {
                "xT": np.ascontiguousarray(x[b].T).astype(NPBF16),
                "wqkvT": np.ascontiguousarray(W1.T).astype(NPBF16),
                "wpT": np.ascontiguousarray(Wp.T).astype(NPBF16),
                "bqk": bqk,
                "bv": bv,
                "bo": bo,
            }
        )
    return in_maps


def kernel(**inputs):
    if "nc" not in _NC_CACHE:
        _NC_CACHE["nc"] = _build()
    nc = _NC_CACHE["nc"]

    res = run_bass_kernel_spmd(nc, _in_maps(inputs), core_ids=list(range(NCORES)))
    out = np.stack([r["out"] for r in res.results], axis=0)
    return out.astype(np.float32)
